# revision 33
# baseline (speedup 1.0000x reference)
"""Trainium2 Bass kernel: 4-layer GPT-2-style transformer (B=2, S=2048, D=1024,
H=16, DH=64, M=4096, V=50257) on 8 NeuronCores.

Sharding (one SPMD program, no core-dependent control flow or addressing):
  - Residual stream / LN / MLP / W_O: sequence-parallel. Core c owns batch
    b = c//4, tokens [512*g, 512*(g+1)) with g = c%4.
  - Attention: head-parallel. Core c computes heads {2c, 2c+1} for BOTH
    batches and all tokens. Per layer: one 8-core AllGather of x_ln^T and
    one 8-core AllToAll routing z back from head-shards to token-shards.
  - Unembed: vocab-parallel. 8-core AllGather of final^T, then every core
    computes all 4096 tokens x its 6283-column vocab shard.
  - All matmuls use fp16 operands with f32 PSUM accumulation (full PE rate
    at any moving-dim, 16-bit LDWEIGHTS). Residual stream and LN/softmax
    statistics stay f32/f32r.
  - Weights are pre-swizzled on the host into [partition, k, cols] layouts
    (contiguous per-partition DMA) and spread across the three DMA-capable
    queues: sync=activations, scalar=small weights, gpsimd=big weights.
"""

import sys, os
sys.path.insert(0, '/opt/trn_rl_repo')
os.environ.setdefault('MYCRO_LOCAL_CACHE', '1')

from contextlib import ExitStack

import numpy as np

import concourse.bass as bass
import concourse.bacc as bacc
import concourse.mybir as mybir
import concourse.tile as tile
from concourse.bass_utils import run_bass_kernel_spmd
from concourse.masks import make_identity

# model dims
B, S, V, D, H, DH, MLPD, L = 2, 2048, 50257, 1024, 16, 64, 4096, 4
EPS = 1e-5
NCORES = 8
G = 4                 # sequence-parallel degree within a batch
T = S // G            # 512 local tokens per core
BS = B * S            # 4096 total tokens
DT = D // 128         # 8 d-tiles
INV_SQRT_DH = float(1.0 / np.sqrt(DH))
VSH = (V + NCORES - 1) // NCORES      # 6283 true vocab shard width
MASK_NEG = -30000.0                   # f16-safe; exp(scale*(s+MASK_NEG)) == 0

F32 = mybir.dt.float32
F32R = mybir.dt.float32r
I32 = mybir.dt.int32
F16 = mybir.dt.float16
AF = mybir.ActivationFunctionType
OP = mybir.AluOpType

ALL8 = [[0, 1, 2, 3, 4, 5, 6, 7]]

_COMPILED = None


def ts(i, n):
    return slice(i * n, (i + 1) * n)


def _build():
    nc = bacc.Bacc("TRN2", target_bir_lowering=False, debug=False,
                   num_devices=NCORES)

    # ---------------- I/O -----------------
    tok_d = nc.dram_tensor("tok", [T], I32, kind="ExternalInput")
    we_d = nc.dram_tensor("we", [V, D], F16, kind="ExternalInput")
    wpos_d = nc.dram_tensor("wposT", [D, T], F32, kind="ExternalInput")
    # per-core head slice: q-pair (128) | k-pair (128), pre-swizzled
    wqk_d = nc.dram_tensor("wqk", [L, 128, DT, 256], F16, kind="ExternalInput")
    wv_d = nc.dram_tensor("wv", [L, 128, DT, 128], F16, kind="ExternalInput")
    wo_d = nc.dram_tensor("wo", [L, 128, DT, D], F16, kind="ExternalInput")
    wi_d = nc.dram_tensor("wi", [L, 4, 128, DT, 1024], F16,
                          kind="ExternalInput")
    wout_d = nc.dram_tensor("wout", [L, 4, 128, 8, 1024], F16,
                            kind="ExternalInput")
    wu_d = nc.dram_tensor("wu", [128, DT, VSH], F16, kind="ExternalInput")
    logits_d = nc.dram_tensor("logits", [BS, VSH], F16, kind="ExternalOutput")

    # ------------- collective buffers -------------
    # x_ln / final bounces split in d-halves so the second AllGather half
    # overlaps the first half's consumers
    xbh = [nc.dram_tensor(f"xb{h}", [128, 4, T], F16) for h in range(2)]
    xgh = [nc.dram_tensor(f"xg{h}", [NCORES, 128, 4, T], F16,
                          addr_space="Shared") for h in range(2)]
    zb = nc.dram_tensor("zb", [NCORES, 128, T], F16)      # z bounce (A2A in)
    zg = nc.dram_tensor("zg", [NCORES, 128, T], F16)      # A2A out
    fbh = [nc.dram_tensor(f"fb{h}", [128, 4, T], F16) for h in range(2)]
    fgh = [nc.dram_tensor(f"fg{h}", [NCORES, 128, 4, T], F16,
                          addr_space="Shared") for h in range(2)]

    with tile.TileContext(nc) as tc:
        # PSUM pools: sc 2x(2 banks) + z 2x(1) + v 2x(1) = 8 banks.
        with tc.tile_pool(name="ps_sc", bufs=2, space="PSUM") as pps_sc, \
             tc.tile_pool(name="ps_z", bufs=2, space="PSUM") as pps_z, \
             tc.tile_pool(name="ps_v", bufs=2, space="PSUM") as pps_v:

            with ExitStack() as lctx:
                pc = lctx.enter_context(tc.tile_pool(name="const", bufs=1))
                pscr = lctx.enter_context(tc.tile_pool(name="scr", bufs=2))
                pln = lctx.enter_context(tc.tile_pool(name="ln", bufs=2))
                pst = lctx.enter_context(tc.tile_pool(name="stats", bufs=1))
                presid = lctx.enter_context(tc.tile_pool(name="resid", bufs=8))
                pxln = lctx.enter_context(tc.tile_pool(name="xln", bufs=8))
                pxg = lctx.enter_context(tc.tile_pool(name="xgch", bufs=2))
                pbig = lctx.enter_context(tc.tile_pool(name="big", bufs=2))
                pvaug = lctx.enter_context(tc.tile_pool(name="vaug", bufs=4))
                pvt = lctx.enter_context(tc.tile_pool(name="vt", bufs=2))
                pex = lctx.enter_context(tc.tile_pool(name="ex", bufs=2))
                pzu = lctx.enter_context(tc.tile_pool(name="zu", bufs=1))
                pzc = lctx.enter_context(tc.tile_pool(name="zc", bufs=2))
                pxe = lctx.enter_context(tc.tile_pool(name="xe", bufs=1))
                pwqk = lctx.enter_context(tc.tile_pool(name="wqk", bufs=1))
                pwv = lctx.enter_context(tc.tile_pool(name="wv", bufs=1))
                pwo = lctx.enter_context(tc.tile_pool(name="wo", bufs=1))
                pwi = lctx.enter_context(tc.tile_pool(name="wi", bufs=2))
                pwout = lctx.enter_context(tc.tile_pool(name="wout", bufs=2))
                ppost = lctx.enter_context(tc.tile_pool(name="post", bufs=8))

                # ---------- constants ----------
                ident = pscr.tile([128, 128], F32, tag="ident", bufs=1)
                make_identity(nc, ident[:])
                identh = pc.tile([128, 128], F16, tag="identh")
                nc.vector.tensor_copy(identh[:], ident[:])
                onesf = pscr.tile([128, 128], F32, tag="onesf", bufs=1)
                nc.vector.memset(onesf[:], 1.0)
                ones_c = pc.tile([128, 1], F32R, tag="ones_c")
                nc.vector.tensor_copy(ones_c[:], onesf[:, 0:1])
                ones_r64 = pc.tile([1, 64], F16, tag="ones_r64")
                nc.vector.tensor_copy(ones_r64[:], onesf[0:1, 0:64])
                ones_r128 = pc.tile([1, 128], F32R, tag="ones_r128")
                nc.vector.tensor_copy(ones_r128[:], onesf[0:1, :])
                eps_t = pc.tile([1, 1], F32, tag="eps")
                nc.vector.memset(eps_t[:], EPS)
                # additive causal masks for the four 128-key tiles of a
                # diagonal 128x512 chunk; mask_j[k, q] = 0 iff q >= k + 128j
                masks = []
                for j in range(4):
                    mk = pc.tile([128, 512], F16, tag=f"mask{j}")
                    nc.gpsimd.memset(mk[:], 0.0)
                    nc.gpsimd.affine_select(
                        out=mk[:], in_=mk[:], compare_op=OP.is_ge,
                        fill=MASK_NEG, base=-128 * j, pattern=[[1, 512]],
                        channel_multiplier=-1)
                    masks.append(mk)

                # residual stream x^T, [D on partitions, T tokens], f32r
                resid = [presid.tile([128, T], F32R, tag="resid",
                                     name=f"resid{i}")
                         for i in range(DT)]

                def layer_norm(src_tiles, dst_tiles):
                    """dst = (src - mean_D) / sqrt(var_D + eps) per token;
                    x^T layout, stats over the partition (D) axis via
                    ones-matmuls. dst tiles are f16."""
                    sum_ps = pps_v.tile([1, T], F32, tag="v")
                    sq_ps = pps_v.tile([1, T], F32, tag="v")
                    for d in range(DT):
                        sq = pln.tile([128, T], F32R, tag="lnsq")
                        nc.scalar.activation(sq[:], src_tiles[d][:],
                                             AF.Square)
                        nc.tensor.matmul(sum_ps[:], ones_c[:],
                                         src_tiles[d][:],
                                         start=(d == 0), stop=(d == DT - 1))
                        nc.tensor.matmul(sq_ps[:], ones_c[:], sq[:],
                                         start=(d == 0), stop=(d == DT - 1))
                    mean = pst.tile([1, T], F32R, tag="mean")
                    nc.scalar.mul(mean[:], sum_ps[:], 1.0 / D)
                    ems = pst.tile([1, T], F32, tag="ems")
                    nc.scalar.mul(ems[:], sq_ps[:], 1.0 / D)
                    m2 = pst.tile([1, T], F32, tag="m2")
                    nc.scalar.activation(m2[:], mean[:], AF.Square)
                    nc.vector.tensor_tensor(out=ems[:], in0=ems[:],
                                            in1=m2[:], op=OP.subtract)
                    std = pst.tile([1, T], F32, tag="std")
                    nc.scalar.activation(std[:], ems[:], AF.Sqrt,
                                         bias=eps_t[:])
                    rsf = pst.tile([1, T], F32, tag="rcf", bufs=2)
                    nc.vector.reciprocal_approx_fast(rsf[:], std[:])
                    rstd = pst.tile([1, T], F32R, tag="rstd")
                    nc.vector.tensor_copy(rstd[:], rsf[:])
                    bc_m = pps_z.tile([128, T], F32, tag="z")
                    nc.tensor.matmul(bc_m[:], ones_r128[:], mean[:],
                                     start=True, stop=True)
                    bc_r = pps_z.tile([128, T], F32, tag="z")
                    nc.tensor.matmul(bc_r[:], ones_r128[:], rstd[:],
                                     start=True, stop=True)
                    for d in range(DT):
                        tmp = pln.tile([128, T], F16, tag="lntmp")
                        nc.vector.tensor_tensor(out=tmp[:],
                                                in0=src_tiles[d][:],
                                                in1=bc_m[:], op=OP.subtract)
                        nc.vector.tensor_tensor(out=dst_tiles[d][:],
                                                in0=tmp[:], in1=bc_r[:],
                                                op=OP.mult)

                # ================= embedding =================
                with nc.named_scope("embed"):
                    for t in range(T // 128):
                        it = pscr.tile([128, 1], I32, tag="idx")
                        nc.sync.dma_start(
                            it[:],
                            tok_d[ts(t, 128)].rearrange("(p o) -> p o", o=1))
                        xe = pxe.tile([128, D], F16, tag="xe")
                        nc.gpsimd.indirect_dma_start(
                            out=xe[:], out_offset=None, in_=we_d[:],
                            in_offset=bass.IndirectOffsetOnAxis(
                                ap=it[:, :1], axis=0))
                        for d in range(DT):
                            tp = pps_z.tile([128, 128], F16, tag="z")
                            nc.tensor.transpose(tp[:], xe[:, ts(d, 128)],
                                                identh[:])
                            wp = pscr.tile([128, 128], F32, tag="wp")
                            nc.scalar.dma_start(
                                wp[:], wpos_d[ts(d, 128), ts(t, 128)])
                            nc.vector.tensor_tensor(
                                out=resid[d][:, ts(t, 128)], in0=tp[:],
                                in1=wp[:], op=OP.add)

                # ================= layers =================
                for l in range(L):
                    # ---- LN1 + 8-core AllGather of x_ln^T ----
                    with nc.named_scope(f"l{l}_ln1"):
                        xln = [pxln.tile([128, T], F16, tag="xln",
                                         name=f"xln_{l}_{i}")
                               for i in range(DT)]
                        layer_norm(resid, xln)
                        for h in range(2):
                            for d in range(4):
                                nc.sync.dma_start(xbh[h][:, d, :],
                                                  xln[4 * h + d][:])
                            nc.gpsimd.collective_compute(
                                "AllGather", OP.bypass, replica_groups=ALL8,
                                ins=[xbh[h][:]], outs=[xgh[h][:]])

                    # per-layer weight tiles; triggers early, deps already
                    # satisfied (prev layer's reads done) so the issuing
                    # sequencers never stall here.
                    wqk_t = pwqk.tile([128, DT, 256], F16, tag="wqk")
                    nc.scalar.dma_start(wqk_t[:], wqk_d[l])
                    wv_t = pwv.tile([128, DT, 128], F16, tag="wv")
                    nc.scalar.dma_start(wv_t[:], wv_d[l])
                    wo_t = pwo.tile([128, DT, D], F16, tag="wo")
                    nc.gpsimd.dma_start(wo_t[:], wo_d[l])
                    wi_ts = []
                    wout_ts = []
                    for qtr in range(2):   # first two quarters prefetch
                        w1 = pwi.tile([128, DT, 1024], F16, tag="wi",
                                      name=f"wi{l}_{qtr}")
                        nc.gpsimd.dma_start(w1[:], wi_d[l, qtr])
                        wi_ts.append(w1)
                        w2 = pwout.tile([128, 8, 1024], F16, tag="wout",
                                        name=f"wout{l}_{qtr}")
                        nc.gpsimd.dma_start(w2[:], wout_d[l, qtr])
                        wout_ts.append(w2)

                    # ---- q/k/v for my 2 heads over ALL 4096 tokens ----
                    with nc.named_scope(f"l{l}_qkv"):
                        # [128 = 2 heads x 64dh, 4096 tokens]
                        qhp = pbig.tile([128, BS], F16, tag="big",
                                        name=f"qhp{l}")
                        khp = pbig.tile([128, BS], F16, tag="big",
                                        name=f"khp{l}")
                        # v in normal layout + ones column:
                        # [128 tok, 8 keytiles, 2 heads, 64+1], per (b2, grp)
                        vaug = [[pvaug.tile([128, 8, 2, 65], F16, tag="vaug",
                                            name=f"vaug{l}_{b2}_{g}")
                                 for g in range(2)] for b2 in range(2)]
                        for b2 in range(2):
                            for g in range(2):
                                nc.vector.memset(
                                    vaug[b2][g][:, :, :, 64:65], 1.0)
                        for tc8 in range(8):          # 512-token chunks
                            xga = pxg.tile([128, 4, T], F16, tag="xg",
                                           name=f"xga{l}_{tc8}")
                            nc.sync.dma_start(xga[:], xgh[0][tc8])
                            xgb = pxg.tile([128, 4, T], F16, tag="xg2",
                                           name=f"xgb{l}_{tc8}")
                            nc.sync.dma_start(xgb[:], xgh[1][tc8])
                            xk = lambda k: (xga[:, k, :] if k < 4
                                            else xgb[:, k - 4, :])
                            for m in range(2):        # q pair, k pair
                                ps = pps_sc.tile([128, T], F32, tag="sc")
                                for k in range(DT):
                                    nc.tensor.matmul(
                                        ps[:], wqk_t[:, k, ts(m, 128)],
                                        xk(k),
                                        start=(k == 0), stop=(k == DT - 1))
                                dst = qhp if m == 0 else khp
                                nc.scalar.copy(dst[:, ts(tc8, T)], ps[:])
                            # v^T then PE-transpose to normal layout
                            psv = pps_sc.tile([128, T], F32, tag="sc")
                            for k in range(DT):
                                nc.tensor.matmul(
                                    psv[:], wv_t[:, k, :], xk(k),
                                    start=(k == 0), stop=(k == DT - 1))
                            vt = pvt.tile([128, T], F16, tag="vt",
                                          name=f"vt{l}_{tc8}")
                            nc.scalar.copy(vt[:], psv[:])
                            for tt in range(4):
                                g32 = 4 * tc8 + tt    # global 128-key tile
                                tp = pps_z.tile([128, 128], F16, tag="z")
                                nc.tensor.transpose(
                                    tp[:], vt[:, ts(tt, 128)], identh[:])
                                b2, k16 = g32 // 16, g32 % 16
                                nc.vector.tensor_copy(
                                    vaug[b2][k16 // 8][:, k16 % 8, :, 0:64],
                                    tp[:].rearrange("p (h c) -> p h c", h=2))

                    # ---- attention: 2 heads x 2 batches, all queries ----
                    # scores over kt pairs into [128,1024] PSUM, one EXP per
                    # pair (halves the per-activation overhead)
                    with nc.named_scope(f"l{l}_attn"):
                        for b2 in range(2):
                            for qc in range(4):       # 512-query chunks
                                cb = 2048 * b2 + 512 * qc
                                nk = 4 * (qc + 1)
                                zps = [pps_z.tile([65, 512], F32, tag="z",
                                                  name=f"zps{hh}")
                                       for hh in range(2)]
                                for ktp in range(nk // 2):
                                    kt0, kt1 = 2 * ktp, 2 * ktp + 1
                                    for hh in range(2):
                                        scps = pps_sc.tile([128, 1024], F32,
                                                           tag="sc",
                                                           name="scps")
                                        for half, kt in ((0, kt0), (1, kt1)):
                                            nc.tensor.matmul(
                                                scps[:, ts(half, 512)],
                                                khp[ts(hh, 64),
                                                    2048 * b2 + 128 * kt:
                                                    2048 * b2
                                                    + 128 * (kt + 1)],
                                                qhp[ts(hh, 64), cb:cb + 512],
                                                start=True, stop=True)
                                            if kt >= 4 * qc:
                                                nc.vector.tensor_tensor(
                                                    out=scps[:, ts(half,
                                                                   512)],
                                                    in0=scps[:, ts(half,
                                                                   512)],
                                                    in1=masks[kt - 4 * qc][:],
                                                    op=OP.add)
                                        ex = pex.tile([128, 1024], F16,
                                                      tag="ex")
                                        nc.scalar.activation(
                                            ex[:], scps[:], AF.Exp,
                                            scale=INV_SQRT_DH)
                                        for half, kt in ((0, kt0), (1, kt1)):
                                            nc.tensor.matmul(
                                                zps[hh][:],
                                                vaug[b2][kt // 8][:, kt % 8,
                                                                  hh, 0:65],
                                                ex[:, ts(half, 512)],
                                                start=(kt == 0),
                                                stop=(kt == nk - 1))
                                zc = pzc.tile([128, 512], F16, tag="zc")
                                for hh in range(2):
                                    zu = pzu.tile([64, 512], F16, tag="zu")
                                    nc.scalar.copy(zu[:], zps[hh][0:64, :])
                                    # bounce the denominator row to SBUF
                                    # partition 0: reciprocal_approx_fast
                                    # reads garbage from offset partitions
                                    dn = pst.tile([1, 512], F32, tag="dn",
                                                  bufs=1)
                                    nc.vector.tensor_copy(
                                        dn[:], zps[hh][64:65, :])
                                    rcf = pst.tile([1, 512], F32, tag="rcf",
                                                   bufs=2)
                                    nc.vector.reciprocal_approx_fast(
                                        rcf[:], dn[:])
                                    rc = pst.tile([1, 512], F16, tag="rc",
                                                  bufs=1)
                                    nc.vector.tensor_copy(rc[:], rcf[:])
                                    bc = pps_v.tile([64, 512], F32, tag="v")
                                    nc.tensor.matmul(bc[:], ones_r64[:],
                                                     rc[:], start=True,
                                                     stop=True)
                                    nc.vector.tensor_tensor(
                                        out=zc[ts(hh, 64), :],
                                        in0=zu[:], in1=bc[:], op=OP.mult)
                                nc.sync.dma_start(zb[4 * b2 + qc], zc[:])

                    # ---- z AllToAll (head-shard -> token-shard) + W_O ----
                    with nc.named_scope(f"l{l}_wo"):
                        nc.gpsimd.collective_compute(
                            "AllToAll", OP.bypass, replica_groups=ALL8,
                            ins=[zb[:]], outs=[zg[:]])
                        zgt = []
                        for k in range(DT):
                            zch = pxg.tile([128, T], F16, tag="zg",
                                           name=f"zg{l}_{k}", bufs=8)
                            nc.sync.dma_start(zch[:], zg[k])
                            zgt.append(zch)
                        for m in range(DT):
                            ps = pps_sc.tile([128, T], F32, tag="sc")
                            for k in range(DT):
                                nc.tensor.matmul(
                                    ps[:], wo_t[:, k, ts(m, 128)], zgt[k][:],
                                    start=(k == 0), stop=(k == DT - 1))
                            nc.vector.tensor_tensor(out=resid[m][:],
                                                    in0=resid[m][:],
                                                    in1=ps[:], op=OP.add)

                    # ---- LN2 + MLP ----
                    with nc.named_scope(f"l{l}_mlp"):
                        xln2 = [pxln.tile([128, T], F16, tag="xln",
                                          name=f"xln2_{l}_{i}")
                                for i in range(DT)]
                        layer_norm(resid, xln2)
                        for qtr in range(4):
                            if qtr >= 2:   # stream in the later quarters
                                wi_t = pwi.tile([128, DT, 1024], F16,
                                                tag="wi",
                                                name=f"wi{l}_{qtr}")
                                nc.gpsimd.dma_start(wi_t[:], wi_d[l, qtr])
                                wout_t = pwout.tile([128, 8, 1024], F16,
                                                    tag="wout",
                                                    name=f"wout{l}_{qtr}")
                                nc.gpsimd.dma_start(wout_t[:],
                                                    wout_d[l, qtr])
                            else:
                                wi_t = wi_ts[qtr]
                                wout_t = wout_ts[qtr]
                            post = []
                            for mh in range(8):
                                ps = pps_sc.tile([128, T], F32, tag="sc")
                                for k in range(DT):
                                    nc.tensor.matmul(
                                        ps[:], wi_t[:, k, ts(mh, 128)],
                                        xln2[k][:],
                                        start=(k == 0), stop=(k == DT - 1))
                                po = ppost.tile([128, T], F16, tag="post",
                                                name=f"post{l}_{qtr}_{mh}")
                                nc.scalar.activation(po[:], ps[:],
                                                     AF.Gelu_apprx_tanh)
                                post.append(po)
                            for m in range(DT):
                                ps = pps_sc.tile([128, T], F32, tag="sc")
                                for k in range(8):
                                    nc.tensor.matmul(
                                        ps[:], wout_t[:, k, ts(m, 128)],
                                        post[k][:],
                                        start=(k == 0), stop=(k == 7))
                                nc.vector.tensor_tensor(out=resid[m][:],
                                                        in0=resid[m][:],
                                                        in1=ps[:],
                                                        op=OP.add)

                # ---- final LN + 8-core gather ----
                with nc.named_scope("final_ln"):
                    xf = [pxln.tile([128, T], F16, tag="xln",
                                    name=f"xf{i}")
                          for i in range(DT)]
                    layer_norm(resid, xf)
                    for h in range(2):
                        for d in range(4):
                            nc.sync.dma_start(fbh[h][:, d, :],
                                              xf[4 * h + d][:])
                        nc.gpsimd.collective_compute(
                            "AllGather", OP.bypass, replica_groups=ALL8,
                            ins=[fbh[h][:]], outs=[fgh[h][:]])

            # ================= unembed (vocab shard) =================
            with nc.named_scope("unembed"), \
                 tc.tile_pool(name="uf", bufs=8) as puf, \
                 tc.tile_pool(name="uw", bufs=1) as puw, \
                 tc.tile_pool(name="uo", bufs=3) as puo:
                wu_t = puw.tile([128, DT, VSH], F16, tag="wu")
                nc.scalar.dma_start(wu_t[:], wu_d[:])
                fbl = []
                for blk in range(NCORES):
                    fa = puf.tile([128, 4, T], F16, tag="ft",
                                  name=f"fta{blk}")
                    fb_ = puf.tile([128, 4, T], F16, tag="ft2",
                                   name=f"ftb{blk}")
                    eng = nc.sync if blk % 2 == 0 else nc.gpsimd
                    eng.dma_start(fa[:], fgh[0][blk])
                    eng.dma_start(fb_[:], fgh[1][blk])
                    fbl.append((fa, fb_))
                ntiles = [(512 * i, 512) for i in range(12)] + [(6144, 139)]
                pools3 = [pps_sc, pps_z, pps_v]
                tags3 = ["sc", "z", "v"]
                for tt in range(BS // 128):
                    blk, tl = tt // 4, tt % 4
                    obig = puo.tile([128, VSH], F16, tag="ob")
                    for i, (off, w) in enumerate(ntiles):
                        ps = pools3[i % 3].tile([128, w], F32,
                                                tag=tags3[i % 3])
                        for k in range(DT):
                            fk = (fbl[blk][0][:, k, ts(tl, 128)] if k < 4
                                  else fbl[blk][1][:, k - 4, ts(tl, 128)])
                            nc.tensor.matmul(
                                ps[:], fk, wu_t[:, k, off:off + w],
                                start=(k == 0), stop=(k == DT - 1))
                        if i % 2 == 0:
                            nc.scalar.copy(obig[:, off:off + w], ps[:])
                        else:
                            nc.vector.tensor_copy(obig[:, off:off + w], ps[:])
                    eng = nc.sync if tt % 2 == 0 else nc.gpsimd
                    eng.dma_start(logits_d[ts(tt, 128), :], obig[:])

    nc.compile()
    return nc


def _prep_inputs(inputs):
    """Validate + build the 8 per-core input maps (host-side sharding)."""
    inp = {k: np.asarray(v) for k, v in inputs.items()}

    for name in ('b_Q', 'b_K', 'b_V', 'b_O', 'b_in', 'b_out', 'b_U',
                 'ln1_b', 'ln2_b', 'lnf_b'):
        if inp[name].any():
            raise NotImplementedError(f"nonzero {name} not supported")
    for name in ('ln1_w', 'ln2_w', 'lnf_w'):
        if not np.all(inp[name] == 1.0):
            raise NotImplementedError(f"non-unit {name} not supported")

    tokens = inp['tokens'].astype(np.int32)                      # [B, S]
    WE = np.ascontiguousarray(inp['W_E'], np.float16)            # [V, D]
    W_posT = np.ascontiguousarray(inp['W_pos'].T, np.float32)    # [D, S]
    WQ, WK, WV = inp['W_Q'], inp['W_K'], inp['W_V']              # [L,H,D,DH]
    # W_O [L,H,DH,D] -> [L, HDH, D] -> [L,128,8,D]
    WO = inp['W_O'].reshape(L, H * DH, D)
    WO = np.ascontiguousarray(
        WO.reshape(L, DT, 128, D).transpose(0, 2, 1, 3), np.float16)
    # W_in [L,D,M] -> [L,128,8,M] -> quarters [L,4,128,8,1024]
    WI = inp['W_in'].reshape(L, DT, 128, MLPD).transpose(0, 2, 1, 3)
    WI = np.ascontiguousarray(
        WI.reshape(L, 128, DT, 4, 1024).transpose(0, 3, 1, 2, 4), np.float16)
    # W_out [L,M,D] -> [L,128,32,D] -> quarters [L,4,128,8,1024]
    WOUT = inp['W_out'].reshape(L, 32, 128, D).transpose(0, 2, 1, 3)
    WOUT = np.ascontiguousarray(
        WOUT.reshape(L, 128, 4, 8, D).transpose(0, 2, 1, 3, 4), np.float16)
    WU = inp['W_U'].astype(np.float32)                           # [D, V]

    in_maps = []
    for c in range(NCORES):
        b, g = c // G, c % G
        hsel = slice(2 * c, 2 * c + 2)
        # [L, 2, D, DH] -> [L, D, 2*DH]
        wq_c = WQ[:, hsel].transpose(0, 2, 1, 3).reshape(L, D, 2 * DH)
        wk_c = WK[:, hsel].transpose(0, 2, 1, 3).reshape(L, D, 2 * DH)
        wqk_c = np.concatenate([wq_c, wk_c], axis=2)             # [L,D,256]
        wqk_c = np.ascontiguousarray(
            wqk_c.reshape(L, DT, 128, 256).transpose(0, 2, 1, 3), np.float16)
        wv_c = WV[:, hsel].transpose(0, 2, 1, 3).reshape(L, D, 2 * DH)
        wv_c = np.ascontiguousarray(
            wv_c.reshape(L, DT, 128, 128).transpose(0, 2, 1, 3), np.float16)
        lo, hi = VSH * c, min(VSH * (c + 1), V)
        wu_c = np.zeros((D, VSH), np.float32)
        wu_c[:, :hi - lo] = WU[:, lo:hi]
        wu_c = np.ascontiguousarray(
            wu_c.reshape(DT, 128, VSH).transpose(1, 0, 2), np.float16)
        in_maps.append({
            'tok': np.ascontiguousarray(tokens[b, ts(g, T)]),
            'we': WE,
            'wposT': np.ascontiguousarray(W_posT[:, ts(g, T)]),
            'wqk': wqk_c,
            'wv': wv_c,
            'wo': WO,
            'wi': WI,
            'wout': WOUT,
            'wu': wu_c,
        })
    return in_maps


def kernel(**inputs):
    global _COMPILED
    if _COMPILED is None:
        _COMPILED = _build()
    nc = _COMPILED

    in_maps = _prep_inputs(inputs)
    trace = bool(int(os.environ.get('KERNEL_TRACE', '0')))
    res = run_bass_kernel_spmd(nc, in_maps, core_ids=list(range(NCORES)),
                               trace=trace)
    kernel.last_results = res

    logits = np.empty((B, S, V), np.float32)
    for c in range(NCORES):
        lg = res.results[c]['logits']                 # [4096, VSH] f16
        lo = VSH * c
        hi = min(VSH * (c + 1), V)
        if hi <= lo:
            continue
        for blk in range(NCORES):
            bb, gg = blk // G, blk % G
            logits[bb, ts(gg, T), lo:hi] = lg[ts(blk, T), :hi - lo]
    return logits


# revision 34
# speedup vs baseline: 1.0789x; 1.0789x over previous
"""Trainium2 Bass kernel: 4-layer GPT-2-style transformer (B=2, S=2048, D=1024,
H=16, DH=64, M=4096, V=50257) on 8 NeuronCores.

Sharding (one SPMD program, no core-dependent control flow or addressing):
  - Residual stream / LN / MLP / W_O: sequence-parallel. Core c owns batch
    b = c//4, tokens [512*g, 512*(g+1)) with g = c%4.
  - Attention: head-parallel. Core c computes heads {2c, 2c+1} for BOTH
    batches and all tokens. Per layer: one 8-core AllGather of x_ln^T and
    one 8-core AllToAll routing z back from head-shards to token-shards.
  - Unembed: vocab-parallel. 8-core AllGather of final^T, then every core
    computes all 4096 tokens x its 6283-column vocab shard.
  - All matmuls use fp16 operands with f32 PSUM accumulation (full PE rate
    at any moving-dim, 16-bit LDWEIGHTS). Residual stream and LN/softmax
    statistics stay f32/f32r.
  - Weights are pre-swizzled on the host into [partition, k, cols] layouts
    (contiguous per-partition DMA) and spread across the three DMA-capable
    queues: sync=activations, scalar=small weights, gpsimd=big weights.
"""

import sys, os
sys.path.insert(0, '/opt/trn_rl_repo')
os.environ.setdefault('MYCRO_LOCAL_CACHE', '1')

from contextlib import ExitStack

import numpy as np

import concourse.bass as bass
import concourse.bacc as bacc
import concourse.mybir as mybir
import concourse.tile as tile
from concourse.bass_utils import run_bass_kernel_spmd
from concourse.masks import make_identity

# model dims
B, S, V, D, H, DH, MLPD, L = 2, 2048, 50257, 1024, 16, 64, 4096, 4
EPS = 1e-5
NCORES = 8
G = 4                 # sequence-parallel degree within a batch
T = S // G            # 512 local tokens per core
BS = B * S            # 4096 total tokens
DT = D // 128         # 8 d-tiles
INV_SQRT_DH = float(1.0 / np.sqrt(DH))
VSH = (V + NCORES - 1) // NCORES      # 6283 true vocab shard width
MASK_NEG = -30000.0                   # f16-safe; exp(scale*(s+MASK_NEG)) == 0

F32 = mybir.dt.float32
F32R = mybir.dt.float32r
I32 = mybir.dt.int32
F16 = mybir.dt.float16
AF = mybir.ActivationFunctionType
OP = mybir.AluOpType

ALL8 = [[0, 1, 2, 3, 4, 5, 6, 7]]

_COMPILED = None


def ts(i, n):
    return slice(i * n, (i + 1) * n)


def _build():
    nc = bacc.Bacc("TRN2", target_bir_lowering=False, debug=False,
                   num_devices=NCORES)

    # ---------------- I/O -----------------
    tok_d = nc.dram_tensor("tok", [T], I32, kind="ExternalInput")
    we_d = nc.dram_tensor("we", [V, D], F16, kind="ExternalInput")
    wpos_d = nc.dram_tensor("wposT", [D, T], F32, kind="ExternalInput")
    # per-core head slice: q-pair (128) | k-pair (128), pre-swizzled
    wqk_d = nc.dram_tensor("wqk", [L, 128, DT, 256], F16, kind="ExternalInput")
    wv_d = nc.dram_tensor("wv", [L, 128, DT, 128], F16, kind="ExternalInput")
    wo_d = nc.dram_tensor("wo", [L, 128, DT, D], F16, kind="ExternalInput")
    wi_d = nc.dram_tensor("wi", [L, 4, 128, DT, 1024], F16,
                          kind="ExternalInput")
    wout_d = nc.dram_tensor("wout", [L, 4, 128, 8, 1024], F16,
                            kind="ExternalInput")
    wu_d = nc.dram_tensor("wu", [128, DT, VSH], F16, kind="ExternalInput")
    logits_d = nc.dram_tensor("logits", [BS, VSH], F16, kind="ExternalOutput")

    # ------------- collective buffers -------------
    # x_ln / final bounces split in d-halves so the second AllGather half
    # overlaps the first half's consumers
    xbh = [nc.dram_tensor(f"xb{h}", [128, 4, T], F16) for h in range(2)]
    xgh = [nc.dram_tensor(f"xg{h}", [NCORES, 128, 4, T], F16,
                          addr_space="Shared") for h in range(2)]
    zb = nc.dram_tensor("zb", [NCORES, 128, T], F16)      # z bounce (A2A in)
    zg = nc.dram_tensor("zg", [NCORES, 128, T], F16)      # A2A out
    fbh = [nc.dram_tensor(f"fb{h}", [128, 4, T], F16) for h in range(2)]
    fgh = [nc.dram_tensor(f"fg{h}", [NCORES, 128, 4, T], F16,
                          addr_space="Shared") for h in range(2)]

    with tile.TileContext(nc) as tc:
        # PSUM pools: sc 2x(2 banks) + z 2x(1) + v 2x(1) = 8 banks.
        with tc.tile_pool(name="ps_sc", bufs=2, space="PSUM") as pps_sc, \
             tc.tile_pool(name="ps_z", bufs=2, space="PSUM") as pps_z, \
             tc.tile_pool(name="ps_v", bufs=2, space="PSUM") as pps_v:

            with ExitStack() as lctx:
                pc = lctx.enter_context(tc.tile_pool(name="const", bufs=1))
                pscr = lctx.enter_context(tc.tile_pool(name="scr", bufs=2))
                pln = lctx.enter_context(tc.tile_pool(name="ln", bufs=2))
                pst = lctx.enter_context(tc.tile_pool(name="stats", bufs=1))
                presid = lctx.enter_context(tc.tile_pool(name="resid", bufs=8))
                pxln = lctx.enter_context(tc.tile_pool(name="xln", bufs=8))
                pxg = lctx.enter_context(tc.tile_pool(name="xgch", bufs=2))
                pbig = lctx.enter_context(tc.tile_pool(name="big", bufs=2))
                pvaug = lctx.enter_context(tc.tile_pool(name="vaug", bufs=4))
                pvt = lctx.enter_context(tc.tile_pool(name="vt", bufs=2))
                pex = lctx.enter_context(tc.tile_pool(name="ex", bufs=2))
                pzu = lctx.enter_context(tc.tile_pool(name="zu", bufs=2))
                pzc = lctx.enter_context(tc.tile_pool(name="zc", bufs=2))
                pxe = lctx.enter_context(tc.tile_pool(name="xe", bufs=1))
                pwqk = lctx.enter_context(tc.tile_pool(name="wqk", bufs=1))
                pwv = lctx.enter_context(tc.tile_pool(name="wv", bufs=1))
                pwo = lctx.enter_context(tc.tile_pool(name="wo", bufs=1))
                pwi = lctx.enter_context(tc.tile_pool(name="wi", bufs=2))
                pwout = lctx.enter_context(tc.tile_pool(name="wout", bufs=2))
                ppost = lctx.enter_context(tc.tile_pool(name="post", bufs=8))

                # ---------- constants ----------
                ident = pscr.tile([128, 128], F32, tag="ident", bufs=1)
                make_identity(nc, ident[:])
                identh = pc.tile([128, 128], F16, tag="identh")
                nc.vector.tensor_copy(identh[:], ident[:])
                onesf = pscr.tile([128, 128], F32, tag="onesf", bufs=1)
                nc.vector.memset(onesf[:], 1.0)
                ones_c = pc.tile([128, 1], F32R, tag="ones_c")
                nc.vector.tensor_copy(ones_c[:], onesf[:, 0:1])
                ones_c16 = pc.tile([128, 1], F16, tag="ones_c16")
                nc.vector.tensor_copy(ones_c16[:], onesf[:, 0:1])
                ones_r64 = pc.tile([1, 64], F16, tag="ones_r64")
                nc.vector.tensor_copy(ones_r64[:], onesf[0:1, 0:64])
                ones_r128 = pc.tile([1, 128], F32R, tag="ones_r128")
                nc.vector.tensor_copy(ones_r128[:], onesf[0:1, :])
                eps_t = pc.tile([1, 1], F32, tag="eps")
                nc.vector.memset(eps_t[:], EPS)
                # additive causal masks for the four 128-key tiles of a
                # diagonal 128x512 chunk; mask_j[k, q] = 0 iff q >= k + 128j
                masks = []
                for j in range(4):
                    mk = pc.tile([128, 512], F16, tag=f"mask{j}")
                    nc.gpsimd.memset(mk[:], 0.0)
                    nc.gpsimd.affine_select(
                        out=mk[:], in_=mk[:], compare_op=OP.is_ge,
                        fill=MASK_NEG, base=-128 * j, pattern=[[1, 512]],
                        channel_multiplier=-1)
                    masks.append(mk)

                # residual stream x^T, [D on partitions, T tokens], f32r
                resid = [presid.tile([128, T], F32R, tag="resid",
                                     name=f"resid{i}")
                         for i in range(DT)]

                def layer_norm(src_tiles, dst_tiles):
                    """dst = (src - mean_D) / sqrt(var_D + eps) per token;
                    x^T layout, stats over the partition (D) axis via
                    ones-matmuls. dst tiles are f16."""
                    sum_ps = pps_v.tile([1, T], F32, tag="v")
                    sq_ps = pps_v.tile([1, T], F32, tag="v")
                    for d in range(DT):
                        sq = pln.tile([128, T], F16, tag="lnsq")
                        nc.scalar.activation(sq[:], src_tiles[d][:],
                                             AF.Square)
                        nc.tensor.matmul(sum_ps[:], ones_c[:],
                                         src_tiles[d][:],
                                         start=(d == 0), stop=(d == DT - 1))
                        nc.tensor.matmul(sq_ps[:], ones_c16[:], sq[:],
                                         start=(d == 0), stop=(d == DT - 1))
                    mean = pst.tile([1, T], F32R, tag="mean")
                    nc.scalar.mul(mean[:], sum_ps[:], 1.0 / D)
                    ems = pst.tile([1, T], F32, tag="ems")
                    nc.scalar.mul(ems[:], sq_ps[:], 1.0 / D)
                    m2 = pst.tile([1, T], F32, tag="std")
                    nc.scalar.activation(m2[:], mean[:], AF.Square)
                    nc.vector.tensor_tensor(out=ems[:], in0=ems[:],
                                            in1=m2[:], op=OP.subtract)
                    std = pst.tile([1, T], F32, tag="std")
                    nc.scalar.activation(std[:], ems[:], AF.Sqrt,
                                         bias=eps_t[:])
                    rsf = pst.tile([1, T], F32, tag="rcf", bufs=2)
                    nc.vector.reciprocal_approx_fast(rsf[:], std[:])
                    rstd = pst.tile([1, T], F32R, tag="rstd")
                    nc.vector.tensor_copy(rstd[:], rsf[:])
                    bc_m = pps_z.tile([128, T], F32, tag="z")
                    nc.tensor.matmul(bc_m[:], ones_r128[:], mean[:],
                                     start=True, stop=True)
                    bc_r = pps_z.tile([128, T], F32, tag="z")
                    nc.tensor.matmul(bc_r[:], ones_r128[:], rstd[:],
                                     start=True, stop=True)
                    for d in range(DT):
                        tmp = pln.tile([128, T], F16, tag="lntmp")
                        nc.vector.tensor_tensor(out=tmp[:],
                                                in0=src_tiles[d][:],
                                                in1=bc_m[:], op=OP.subtract)
                        nc.vector.tensor_tensor(out=dst_tiles[d][:],
                                                in0=tmp[:], in1=bc_r[:],
                                                op=OP.mult)

                # ================= embedding =================
                with nc.named_scope("embed"):
                    for t in range(T // 128):
                        it = pscr.tile([128, 1], I32, tag="idx")
                        nc.sync.dma_start(
                            it[:],
                            tok_d[ts(t, 128)].rearrange("(p o) -> p o", o=1))
                        xe = pxe.tile([128, D], F16, tag="xe")
                        nc.gpsimd.indirect_dma_start(
                            out=xe[:], out_offset=None, in_=we_d[:],
                            in_offset=bass.IndirectOffsetOnAxis(
                                ap=it[:, :1], axis=0))
                        for d in range(DT):
                            tp = pps_z.tile([128, 128], F16, tag="z")
                            nc.tensor.transpose(tp[:], xe[:, ts(d, 128)],
                                                identh[:])
                            wp = pscr.tile([128, 128], F32, tag="wp")
                            nc.scalar.dma_start(
                                wp[:], wpos_d[ts(d, 128), ts(t, 128)])
                            nc.vector.tensor_tensor(
                                out=resid[d][:, ts(t, 128)], in0=tp[:],
                                in1=wp[:], op=OP.add)

                # ================= layers =================
                for l in range(L):
                    # ---- LN1 + 8-core AllGather of x_ln^T ----
                    with nc.named_scope(f"l{l}_ln1"):
                        xln = [pxln.tile([128, T], F16, tag="xln",
                                         name=f"xln_{l}_{i}")
                               for i in range(DT)]
                        layer_norm(resid, xln)
                        for h in range(2):
                            for d in range(4):
                                nc.sync.dma_start(xbh[h][:, d, :],
                                                  xln[4 * h + d][:])
                            nc.gpsimd.collective_compute(
                                "AllGather", OP.bypass, replica_groups=ALL8,
                                ins=[xbh[h][:]], outs=[xgh[h][:]])

                    # per-layer weight tiles; triggers early, deps already
                    # satisfied (prev layer's reads done) so the issuing
                    # sequencers never stall here.
                    wqk_t = pwqk.tile([128, DT, 256], F16, tag="wqk")
                    nc.scalar.dma_start(wqk_t[:], wqk_d[l])
                    wv_t = pwv.tile([128, DT, 128], F16, tag="wv")
                    nc.scalar.dma_start(wv_t[:], wv_d[l])
                    wo_t = pwo.tile([128, DT, D], F16, tag="wo")
                    nc.gpsimd.dma_start(wo_t[:], wo_d[l])
                    wi_ts = []
                    wout_ts = []
                    for qtr in range(2):   # first two quarters prefetch
                        w1 = pwi.tile([128, DT, 1024], F16, tag="wi",
                                      name=f"wi{l}_{qtr}")
                        nc.gpsimd.dma_start(w1[:], wi_d[l, qtr])
                        wi_ts.append(w1)
                        w2 = pwout.tile([128, 8, 1024], F16, tag="wout",
                                        name=f"wout{l}_{qtr}")
                        nc.gpsimd.dma_start(w2[:], wout_d[l, qtr])
                        wout_ts.append(w2)

                    # ---- q/k/v for my 2 heads over ALL 4096 tokens ----
                    with nc.named_scope(f"l{l}_qkv"):
                        # [128 = 2 heads x 64dh, 4096 tokens]
                        qhp = pbig.tile([128, BS], F16, tag="big",
                                        name=f"qhp{l}")
                        khp = pbig.tile([128, BS], F16, tag="big",
                                        name=f"khp{l}")
                        # v in normal layout + ones column:
                        # [128 tok, 8 keytiles, 2 heads, 64+1], per (b2, grp)
                        vaug = [[pvaug.tile([128, 8, 2, 65], F16, tag="vaug",
                                            name=f"vaug{l}_{b2}_{g}")
                                 for g in range(2)] for b2 in range(2)]
                        for b2 in range(2):
                            for g in range(2):
                                nc.vector.memset(
                                    vaug[b2][g][:, :, :, 64:65], 1.0)
                        for tc8 in range(8):          # 512-token chunks
                            xga = pxg.tile([128, 4, T], F16, tag="xg",
                                           name=f"xga{l}_{tc8}")
                            nc.sync.dma_start(xga[:], xgh[0][tc8])
                            xgb = pxg.tile([128, 4, T], F16, tag="xg2",
                                           name=f"xgb{l}_{tc8}")
                            nc.sync.dma_start(xgb[:], xgh[1][tc8])
                            xk = lambda k: (xga[:, k, :] if k < 4
                                            else xgb[:, k - 4, :])
                            for m in range(2):        # q pair, k pair
                                ps = pps_sc.tile([128, T], F32, tag="sc")
                                for k in range(DT):
                                    nc.tensor.matmul(
                                        ps[:], wqk_t[:, k, ts(m, 128)],
                                        xk(k),
                                        start=(k == 0), stop=(k == DT - 1))
                                dst = qhp if m == 0 else khp
                                nc.scalar.copy(dst[:, ts(tc8, T)], ps[:])
                            # v^T then PE-transpose to normal layout
                            psv = pps_sc.tile([128, T], F32, tag="sc")
                            for k in range(DT):
                                nc.tensor.matmul(
                                    psv[:], wv_t[:, k, :], xk(k),
                                    start=(k == 0), stop=(k == DT - 1))
                            vt = pvt.tile([128, T], F16, tag="vt",
                                          name=f"vt{l}_{tc8}")
                            nc.scalar.copy(vt[:], psv[:])
                            for tt in range(4):
                                g32 = 4 * tc8 + tt    # global 128-key tile
                                tp = pps_z.tile([128, 128], F16, tag="z")
                                nc.tensor.transpose(
                                    tp[:], vt[:, ts(tt, 128)], identh[:])
                                b2, k16 = g32 // 16, g32 % 16
                                nc.vector.tensor_copy(
                                    vaug[b2][k16 // 8][:, k16 % 8, :, 0:64],
                                    tp[:].rearrange("p (h c) -> p h c", h=2))

                    # ---- attention: 2 heads x 2 batches, all queries ----
                    # scores over kt pairs into [128,1024] PSUM, one EXP per
                    # pair (halves the per-activation overhead)
                    with nc.named_scope(f"l{l}_attn"):
                        for b2 in range(2):
                            for qc in range(4):       # 512-query chunks
                                cb = 2048 * b2 + 512 * qc
                                nk = 4 * (qc + 1)
                                zps = [pps_z.tile([65, 512], F32, tag="z",
                                                  name=f"zps{hh}")
                                       for hh in range(2)]
                                for ktp in range(nk // 2):
                                    kt0, kt1 = 2 * ktp, 2 * ktp + 1
                                    for hh in range(2):
                                        scps = pps_sc.tile([128, 1024], F32,
                                                           tag="sc",
                                                           name="scps")
                                        for half, kt in ((0, kt0), (1, kt1)):
                                            nc.tensor.matmul(
                                                scps[:, ts(half, 512)],
                                                khp[ts(hh, 64),
                                                    2048 * b2 + 128 * kt:
                                                    2048 * b2
                                                    + 128 * (kt + 1)],
                                                qhp[ts(hh, 64), cb:cb + 512],
                                                start=True, stop=True)
                                            if kt >= 4 * qc:
                                                nc.vector.tensor_tensor(
                                                    out=scps[:, ts(half,
                                                                   512)],
                                                    in0=scps[:, ts(half,
                                                                   512)],
                                                    in1=masks[kt - 4 * qc][:],
                                                    op=OP.add)
                                        ex = pex.tile([128, 1024], F16,
                                                      tag="ex")
                                        nc.scalar.activation(
                                            ex[:], scps[:], AF.Exp,
                                            scale=INV_SQRT_DH)
                                        for half, kt in ((0, kt0), (1, kt1)):
                                            nc.tensor.matmul(
                                                zps[hh][:],
                                                vaug[b2][kt // 8][:, kt % 8,
                                                                  hh, 0:65],
                                                ex[:, ts(half, 512)],
                                                start=(kt == 0),
                                                stop=(kt == nk - 1))
                                zc = pzc.tile([128, 512], F16, tag="zc")
                                for hh in range(2):
                                    zu = pzu.tile([64, 512], F16, tag="zu")
                                    nc.scalar.copy(zu[:], zps[hh][0:64, :])
                                    # bounce the denominator row to SBUF
                                    # partition 0: reciprocal_approx_fast
                                    # reads garbage from offset partitions
                                    dn = pst.tile([1, 512], F32, tag="dn",
                                                  bufs=2)
                                    nc.vector.tensor_copy(
                                        dn[:], zps[hh][64:65, :])
                                    rcf = pst.tile([1, 512], F32, tag="rcf",
                                                   bufs=2)
                                    nc.vector.reciprocal_approx_fast(
                                        rcf[:], dn[:])
                                    rc = pst.tile([1, 512], F16, tag="rc",
                                                  bufs=2)
                                    nc.vector.tensor_copy(rc[:], rcf[:])
                                    bc = pps_v.tile([64, 512], F32, tag="v")
                                    nc.tensor.matmul(bc[:], ones_r64[:],
                                                     rc[:], start=True,
                                                     stop=True)
                                    nc.vector.tensor_tensor(
                                        out=zc[ts(hh, 64), :],
                                        in0=zu[:], in1=bc[:], op=OP.mult)
                                nc.sync.dma_start(zb[4 * b2 + qc], zc[:])

                    # ---- z AllToAll (head-shard -> token-shard) + W_O ----
                    with nc.named_scope(f"l{l}_wo"):
                        nc.gpsimd.collective_compute(
                            "AllToAll", OP.bypass, replica_groups=ALL8,
                            ins=[zb[:]], outs=[zg[:]])
                        zgt = []
                        for k in range(DT):
                            zch = pxg.tile([128, T], F16, tag="zg",
                                           name=f"zg{l}_{k}", bufs=8)
                            nc.sync.dma_start(zch[:], zg[k])
                            zgt.append(zch)
                        for m in range(DT):
                            ps = pps_sc.tile([128, T], F32, tag="sc")
                            for k in range(DT):
                                nc.tensor.matmul(
                                    ps[:], wo_t[:, k, ts(m, 128)], zgt[k][:],
                                    start=(k == 0), stop=(k == DT - 1))
                            nc.vector.tensor_tensor(out=resid[m][:],
                                                    in0=resid[m][:],
                                                    in1=ps[:], op=OP.add)

                    # ---- LN2 + MLP ----
                    with nc.named_scope(f"l{l}_mlp"):
                        xln2 = [pxln.tile([128, T], F16, tag="xln",
                                          name=f"xln2_{l}_{i}")
                                for i in range(DT)]
                        layer_norm(resid, xln2)
                        for qtr in range(4):
                            if qtr >= 2:   # stream in the later quarters
                                wi_t = pwi.tile([128, DT, 1024], F16,
                                                tag="wi",
                                                name=f"wi{l}_{qtr}")
                                nc.gpsimd.dma_start(wi_t[:], wi_d[l, qtr])
                                wout_t = pwout.tile([128, 8, 1024], F16,
                                                    tag="wout",
                                                    name=f"wout{l}_{qtr}")
                                nc.gpsimd.dma_start(wout_t[:],
                                                    wout_d[l, qtr])
                            else:
                                wi_t = wi_ts[qtr]
                                wout_t = wout_ts[qtr]
                            post = []
                            for mh in range(8):
                                ps = pps_sc.tile([128, T], F32, tag="sc")
                                for k in range(DT):
                                    nc.tensor.matmul(
                                        ps[:], wi_t[:, k, ts(mh, 128)],
                                        xln2[k][:],
                                        start=(k == 0), stop=(k == DT - 1))
                                po = ppost.tile([128, T], F16, tag="post",
                                                name=f"post{l}_{qtr}_{mh}")
                                nc.scalar.activation(po[:], ps[:],
                                                     AF.Gelu_apprx_tanh)
                                post.append(po)
                            for m in range(DT):
                                ps = pps_sc.tile([128, T], F32, tag="sc")
                                for k in range(8):
                                    nc.tensor.matmul(
                                        ps[:], wout_t[:, k, ts(m, 128)],
                                        post[k][:],
                                        start=(k == 0), stop=(k == 7))
                                nc.vector.tensor_tensor(out=resid[m][:],
                                                        in0=resid[m][:],
                                                        in1=ps[:],
                                                        op=OP.add)

                # ---- final LN + 8-core gather ----
                with nc.named_scope("final_ln"):
                    xf = [pxln.tile([128, T], F16, tag="xln",
                                    name=f"xf{i}")
                          for i in range(DT)]
                    layer_norm(resid, xf)
                    for h in range(2):
                        for d in range(4):
                            nc.sync.dma_start(fbh[h][:, d, :],
                                              xf[4 * h + d][:])
                        nc.gpsimd.collective_compute(
                            "AllGather", OP.bypass, replica_groups=ALL8,
                            ins=[fbh[h][:]], outs=[fgh[h][:]])

            # ================= unembed (vocab shard) =================
            with nc.named_scope("unembed"), \
                 tc.tile_pool(name="uf", bufs=8) as puf, \
                 tc.tile_pool(name="uw", bufs=1) as puw, \
                 tc.tile_pool(name="uo", bufs=3) as puo:
                wu_t = puw.tile([128, DT, VSH], F16, tag="wu")
                nc.scalar.dma_start(wu_t[:], wu_d[:])
                fbl = []
                for blk in range(NCORES):
                    fa = puf.tile([128, 4, T], F16, tag="ft",
                                  name=f"fta{blk}")
                    fb_ = puf.tile([128, 4, T], F16, tag="ft2",
                                   name=f"ftb{blk}")
                    eng = nc.sync if blk % 2 == 0 else nc.gpsimd
                    eng.dma_start(fa[:], fgh[0][blk])
                    eng.dma_start(fb_[:], fgh[1][blk])
                    fbl.append((fa, fb_))
                ntiles = [(512 * i, 512) for i in range(12)] + [(6144, 139)]
                pools3 = [pps_sc, pps_z, pps_v]
                tags3 = ["sc", "z", "v"]
                for tt in range(BS // 128):
                    blk, tl = tt // 4, tt % 4
                    obig = puo.tile([128, VSH], F16, tag="ob")
                    for i, (off, w) in enumerate(ntiles):
                        ps = pools3[i % 3].tile([128, w], F32,
                                                tag=tags3[i % 3])
                        for k in range(DT):
                            fk = (fbl[blk][0][:, k, ts(tl, 128)] if k < 4
                                  else fbl[blk][1][:, k - 4, ts(tl, 128)])
                            nc.tensor.matmul(
                                ps[:], fk, wu_t[:, k, off:off + w],
                                start=(k == 0), stop=(k == DT - 1))
                        if i % 2 == 0:
                            nc.scalar.copy(obig[:, off:off + w], ps[:])
                        else:
                            nc.vector.tensor_copy(obig[:, off:off + w], ps[:])
                    eng = nc.sync if tt % 2 == 0 else nc.gpsimd
                    eng.dma_start(logits_d[ts(tt, 128), :], obig[:])

    nc.compile()
    return nc


def _prep_inputs(inputs):
    """Validate + build the 8 per-core input maps (host-side sharding)."""
    inp = {k: np.asarray(v) for k, v in inputs.items()}

    for name in ('b_Q', 'b_K', 'b_V', 'b_O', 'b_in', 'b_out', 'b_U',
                 'ln1_b', 'ln2_b', 'lnf_b'):
        if inp[name].any():
            raise NotImplementedError(f"nonzero {name} not supported")
    for name in ('ln1_w', 'ln2_w', 'lnf_w'):
        if not np.all(inp[name] == 1.0):
            raise NotImplementedError(f"non-unit {name} not supported")

    tokens = inp['tokens'].astype(np.int32)                      # [B, S]
    WE = np.ascontiguousarray(inp['W_E'], np.float16)            # [V, D]
    W_posT = np.ascontiguousarray(inp['W_pos'].T, np.float32)    # [D, S]
    WQ, WK, WV = inp['W_Q'], inp['W_K'], inp['W_V']              # [L,H,D,DH]
    # W_O [L,H,DH,D] -> [L, HDH, D] -> [L,128,8,D]
    WO = inp['W_O'].reshape(L, H * DH, D)
    WO = np.ascontiguousarray(
        WO.reshape(L, DT, 128, D).transpose(0, 2, 1, 3), np.float16)
    # W_in [L,D,M] -> [L,128,8,M] -> quarters [L,4,128,8,1024]
    WI = inp['W_in'].reshape(L, DT, 128, MLPD).transpose(0, 2, 1, 3)
    WI = np.ascontiguousarray(
        WI.reshape(L, 128, DT, 4, 1024).transpose(0, 3, 1, 2, 4), np.float16)
    # W_out [L,M,D] -> [L,128,32,D] -> quarters [L,4,128,8,1024]
    WOUT = inp['W_out'].reshape(L, 32, 128, D).transpose(0, 2, 1, 3)
    WOUT = np.ascontiguousarray(
        WOUT.reshape(L, 128, 4, 8, D).transpose(0, 2, 1, 3, 4), np.float16)
    WU = inp['W_U'].astype(np.float32)                           # [D, V]

    in_maps = []
    for c in range(NCORES):
        b, g = c // G, c % G
        hsel = slice(2 * c, 2 * c + 2)
        # [L, 2, D, DH] -> [L, D, 2*DH]
        wq_c = WQ[:, hsel].transpose(0, 2, 1, 3).reshape(L, D, 2 * DH)
        wk_c = WK[:, hsel].transpose(0, 2, 1, 3).reshape(L, D, 2 * DH)
        wqk_c = np.concatenate([wq_c, wk_c], axis=2)             # [L,D,256]
        wqk_c = np.ascontiguousarray(
            wqk_c.reshape(L, DT, 128, 256).transpose(0, 2, 1, 3), np.float16)
        wv_c = WV[:, hsel].transpose(0, 2, 1, 3).reshape(L, D, 2 * DH)
        wv_c = np.ascontiguousarray(
            wv_c.reshape(L, DT, 128, 128).transpose(0, 2, 1, 3), np.float16)
        lo, hi = VSH * c, min(VSH * (c + 1), V)
        wu_c = np.zeros((D, VSH), np.float32)
        wu_c[:, :hi - lo] = WU[:, lo:hi]
        wu_c = np.ascontiguousarray(
            wu_c.reshape(DT, 128, VSH).transpose(1, 0, 2), np.float16)
        in_maps.append({
            'tok': np.ascontiguousarray(tokens[b, ts(g, T)]),
            'we': WE,
            'wposT': np.ascontiguousarray(W_posT[:, ts(g, T)]),
            'wqk': wqk_c,
            'wv': wv_c,
            'wo': WO,
            'wi': WI,
            'wout': WOUT,
            'wu': wu_c,
        })
    return in_maps


def kernel(**inputs):
    global _COMPILED
    if _COMPILED is None:
        _COMPILED = _build()
    nc = _COMPILED

    in_maps = _prep_inputs(inputs)
    trace = bool(int(os.environ.get('KERNEL_TRACE', '0')))
    res = run_bass_kernel_spmd(nc, in_maps, core_ids=list(range(NCORES)),
                               trace=trace)
    kernel.last_results = res

    logits = np.empty((B, S, V), np.float32)
    for c in range(NCORES):
        lg = res.results[c]['logits']                 # [4096, VSH] f16
        lo = VSH * c
        hi = min(VSH * (c + 1), V)
        if hi <= lo:
            continue
        for blk in range(NCORES):
            bb, gg = blk // G, blk % G
            logits[bb, ts(gg, T), lo:hi] = lg[ts(blk, T), :hi - lo]
    return logits
